# revision 31
# baseline (speedup 1.0000x reference)
"""Trainium2 Bass kernel for nn_Attention_70557722739202.

Standard MHA block: qkv = x @ Wqkv.T + bqkv; attn = softmax(q k^T / 8);
out = (attn v) @ Wproj.T + bproj, with B=4, N=2048, C=768, H=12, hd=64
(ratio == 1 so the slimmable slicing is identity).

Sharding (8 cores): batch x head-group.  Core c handles batch c//2 and
heads [6*(c%2), 6*(c%2)+6).  Wqkv rows / Wproj cols are sharded by head;
each core emits a partial projection output [2048, 768] and the host sums
the two partials per batch (+ bproj + Wproj @ bv, the exact fold of the
v-bias through the projection).

Rework (425us baseline -> ~300us measured; HW exec time has ~15-18%
run-to-run chip-clock noise, so compare min-of-N):
  - whole datapath in bf16 (PE rate is unchanged vs float32r at >=256
    moving, but SBUF/DMA/DVE traffic and LDWEIGHTS bytes halve); PSUM
    accumulation and the U drains/normalize stay fp32.
  - qkv biases moved off the PE: q/k bias rides the PSUM->SBUF drain as a
    DVE tensor_scalar_add with a per-partition [128,1] bias vector
    (channel-major layout); the v bias is folded on the host into the
    output bias (y += Wproj @ bv exactly, since rows of attn sum to 1).
    This removes 40 K=1 matmuls that each cost a full ~500ns PE pass.
  - both heads' scores share one 2-bank [128,1024] PSUM tile (spool
    bufs=2 = 4 banks, double-buffered so S(i+1) never waits on exp(i)),
    and ONE [128,1024] activation serves the chunk: ScalarE has ~260ns
    fixed per-instruction overhead, so two 512-wide activations are 45%
    slower than one 1024-wide.  PSUM: 4 (S) + 2 (U) + 2 (filler) = 8.
  - startup: single-trigger DMAs (descriptor generation costs ~600ns
    per dma_start), warm-up-critical slices (pair-0 wqk columns, x)
    issued first and split across the sync + scalar queues; only q/k
    column-group 0 + v chunks 0-1 are computed before attention starts,
    everything else runs as PE filler inside the attention chunk loops.
  - tail: proj tiles are emitted as filler per completed j-tile of pair
    2; for s=12..15 the pair-0/1 contributions are precomputed as
    filler and the pair-2 contribution is a split-K=64+64 pair of
    matmuls, head B taken straight from the normalize staging tile so
    the partition-shift DMA (and its ~4us queue drain) stays off the
    critical tail.  After the last normalize only 8 small matmuls +
    DVE adds remain.
  - normalize: broadcast the raw rowsum first (gpsimd), then
    reciprocal_approx_fast at full width on the DVE (a [1,512]
    reciprocal runs on a single DVE lane and is as slow as the
    broadcast itself); normalize-path DMAs ride the gpsimd queue so
    they never wait behind 384KB y-tile writes on the sync queue.
Per-core dataflow is otherwise the baseline's: S.T = k q^T per head via
row-packed K=64 matmuls (2 heads at array rows 0-63/64-127), exp on
ScalarE straight out of PSUM with the 1/8 scale folded in, U.T = [v|1]^T
expS.T accumulated over key chunks (row 64 = softmax denominator), and
the U matmuls of chunk i emitted after chunk i+1's S+exp (software
pipelining that keeps the two row-group S matmuls adjacent).
"""

import os
import sys

for _p in ("/opt/trn_rl_repo",):
    if os.path.isdir(_p) and _p not in sys.path:
        sys.path.insert(0, _p)

import numpy as np

import concourse.bacc as bacc
import concourse.mybir as mybir
import concourse.tile as tile
from concourse.bass_utils import run_bass_kernel_spmd

DIM = 768
NHEADS = 12
B, N = 4, 2048
HD = 64          # head dim
NCORES = 8
HPC = 6          # heads per core
PAIRS = 3        # head pairs per core
GPB = 2          # head groups per batch
CH = HPC * HD    # 384 output channels per core
SCALE = (DIM // NHEADS) ** -0.5
P = 128
QT = 512         # query tile width (1 PSUM bank per S tile)
NJ = N // QT     # 4 query tiles
NKC = N // P     # 16 key chunks
KC = DIM // P    # 6 input-channel chunks
F32 = mybir.dt.float32
F32R = mybir.dt.float32r
BF16 = mybir.dt.bfloat16
EXP = mybir.ActivationFunctionType.Exp

_PROGRAM = None


def _emit(tc, xT_d, wqkT_d, wvT_d, bqk_d, wpT_d, y_d):
    nc = tc.nc

    from contextlib import ExitStack

    with ExitStack() as ctx:
        const = ctx.enter_context(tc.tile_pool(name="const", bufs=1))
        qkpool = ctx.enter_context(tc.tile_pool(name="qkpool", bufs=4))
        atpool = ctx.enter_context(tc.tile_pool(name="atpool", bufs=3))
        epool = ctx.enter_context(tc.tile_pool(name="epool", bufs=4))
        rpool = ctx.enter_context(tc.tile_pool(name="rpool", bufs=2))
        rbpool = ctx.enter_context(tc.tile_pool(name="rbpool", bufs=2))
        uspool = ctx.enter_context(tc.tile_pool(name="uspool", bufs=2))
        ypool = ctx.enter_context(tc.tile_pool(name="ypool", bufs=2))
        spool = ctx.enter_context(tc.tile_pool(name="spool", bufs=2, space="PSUM"))
        upool = ctx.enter_context(tc.tile_pool(name="upool", bufs=2, space="PSUM"))
        fpool = ctx.enter_context(tc.tile_pool(name="fpool", bufs=2, space="PSUM"))

        # ---- resident inputs -------------------------------------------------
        xt = const.tile([P, KC, N], BF16)        # x.T   (in-ch on partitions)
        wqk = const.tile([P, KC, 2 * CH], BF16)  # Wqk.T (in-ch on partitions)
        wv = const.tile([P, KC, CH], BF16)       # Wv.T
        wp = const.tile([P, PAIRS, DIM], BF16)   # Wproj.T slice (ch on part)
        bqk_sb = const.tile([P, 2 * PAIRS], F32)  # col t: q pair t; 3+t: k pair t
        v4 = const.tile([P, NKC, HPC * (HD + 1)], BF16)  # v + ones column

        # Single-trigger transfers (descriptor generation costs ~600ns per
        # dma_start on the issuing engine; per-chunk triggers serialized the
        # stream).  Split across the sync + scalar queues so the two streams
        # move concurrently (~500GB/s aggregate observed).
        nc.sync.dma_start(bqk_sb[:], bqk_d.rearrange("(c p) o -> p (c o)", p=P))
        xtv = xT_d.rearrange("(k p) n -> p k n", p=P)
        wqkv = wqkT_d.rearrange("(k p) n -> p k n", p=P)
        # warm-up-critical first: pair-0's wqk columns + x split across both
        # queues, so the first QKV parts start ~4us sooner; the rest streams
        # behind them.
        nc.scalar.dma_start(wqk[:, :, 0:P], wqkv[:, :, 0:P])
        nc.scalar.dma_start(wqk[:, :, CH:CH + P], wqkv[:, :, CH:CH + P])
        nc.sync.dma_start(xt[:, 0:3, :], xtv[:, 0:3, :])
        nc.gpsimd.dma_start(xt[:, 3:KC, :], xtv[:, 3:KC, :])
        nc.scalar.dma_start(wqk[:, :, P:CH], wqkv[:, :, P:CH])
        nc.scalar.dma_start(wqk[:, :, CH + P:2 * CH], wqkv[:, :, CH + P:2 * CH])
        nc.scalar.dma_start(wv[:], wvT_d.rearrange("(k p) n -> p k n", p=P))
        nc.scalar.dma_start(wp[:], wpT_d.rearrange("(k p) n -> p k n", p=P))
        # pair-2 head-B rows of Wproj.T at base partition 0, for the split-K
        # tail matmul against the normalize staging tile (matmul operands
        # must share a base partition; engines can't shift, DMA can)
        wp2b = const.tile([HD, DIM], BF16)
        nc.sync.dma_start(wp2b[:], wp[HD:P, PAIRS - 1, :])
        v4r = v4.rearrange("p n (h c) -> p n h c", c=HD + 1)
        # softmax-rowsum fused column: ones at channel 64 of each head block
        # (memset can't encode float32r — write through a float32 view)
        ones_cols = v4.rearrange(
            "p n (h c) -> p (n h) c", c=HD + 1
        )[:, :, HD:HD + 1]
        nc.vector.memset(ones_cols, 1.0)

        qk_tiles = {}   # t -> (qt, kt)
        at_tiles = []

        def emit_qkv_pair_part(t, part, nt):
            """One eighth of pair t's q.T/k.T: part in {q,k}, nt in {0..3}
            (512-wide column group).  6 matmuls + one biased drain."""
            if t not in qk_tiles:
                qt_ = qkpool.tile([P, N], BF16, tag="qk", name=f"qt{t}")
                kt_ = qkpool.tile([P, N], BF16, tag="qk", name=f"kt{t}")
                qk_tiles[t] = (qt_, kt_)
            qt_, kt_ = qk_tiles[t]
            colofs = t * P if part == "q" else CH + t * P
            bcol = t if part == "q" else PAIRS + t
            dst = qt_ if part == "q" else kt_
            nsl = slice(nt * QT, (nt + 1) * QT)
            ps = fpool.tile([P, QT], F32, tag="f", name="qkps")
            for k in range(KC):
                nc.tensor.matmul(
                    ps[:],
                    lhsT=wqk[:, k, colofs:colofs + P],
                    rhs=xt[:, k, nsl],
                    start=(k == 0), stop=(k == KC - 1),
                )
            nc.vector.tensor_scalar_add(dst[:, nsl], ps[:], bqk_sb[:, bcol:bcol + 1])

        def emit_v(s):
            """v for all 6 heads for sequence chunk s (bias folded on host)."""
            vps = fpool.tile([P, CH], F32, tag="f", name="vps")
            for k in range(KC):
                nc.tensor.matmul(
                    vps[:],
                    lhsT=xt[:, k, s * P:(s + 1) * P],
                    rhs=wv[:, k, :],
                    start=(k == 0), stop=(k == KC - 1),
                )
            nc.vector.tensor_copy(
                v4r[:, s, :, 0:HD],
                vps.rearrange("p (h c) -> p h c", c=HD),
            )

        def emit_proj_mtile(s):
            """Projection for sequence chunk s: y[s*128:(s+1)*128, :]."""
            ysb = ypool.tile([P, DIM], BF16, tag="y", name="ysb")
            for nh in range(2):
                pps = fpool.tile([P, DIM // 2], F32, tag="f", name="pps")
                for t in range(PAIRS):
                    nc.tensor.matmul(
                        pps[:],
                        lhsT=at_tiles[t][:, s * P:(s + 1) * P],
                        rhs=wp[:, t, nh * (DIM // 2):(nh + 1) * (DIM // 2)],
                        start=(t == 0), stop=(t == PAIRS - 1),
                    )
                nc.vector.tensor_copy(
                    ysb[:, nh * (DIM // 2):(nh + 1) * (DIM // 2)], pps[:]
                )
            nc.sync.dma_start(y_d[s * P:(s + 1) * P, :], ysb[:])

        # the last 4 proj tiles gate on the very last normalize; their
        # pair-0/1 contributions are precomputed as filler so only one
        # matmul + an add per half remains after it.
        proj_partials = {}

        def emit_proj_partial(s):
            yp = ypool.tile([P, DIM], F32, tag="yp", name=f"yp{s}", bufs=4)
            for nh in range(2):
                pps = fpool.tile([P, DIM // 2], F32, tag="f", name="pps")
                for t in range(PAIRS - 1):
                    nc.tensor.matmul(
                        pps[:],
                        lhsT=at_tiles[t][:, s * P:(s + 1) * P],
                        rhs=wp[:, t, nh * (DIM // 2):(nh + 1) * (DIM // 2)],
                        start=(t == 0), stop=(t == PAIRS - 2),
                    )
                nc.vector.tensor_copy(
                    yp[:, nh * (DIM // 2):(nh + 1) * (DIM // 2)], pps[:]
                )
            proj_partials[s] = yp

        def emit_proj_finish(s, stgB):
            """Tail of proj tile s: pair-2 contribution split K=64+K=64 so
            head B comes straight from its normalize staging tile — the
            partition-shift DMA (and its queue drain) stays off the tail."""
            from concourse.alu_op_type import AluOpType

            yp = proj_partials[s]
            scol = s * P - 4 * (NJ - 1) * P
            ysb = ypool.tile([P, DIM], BF16, tag="y", name="ysb")
            for nh in range(2):
                hsl = slice(nh * (DIM // 2), (nh + 1) * (DIM // 2))
                pps = fpool.tile([P, DIM // 2], F32, tag="f", name="pps")
                nc.tensor.matmul(
                    pps[:],
                    lhsT=at_tiles[PAIRS - 1][0:HD, s * P:(s + 1) * P],
                    rhs=wp[0:HD, PAIRS - 1, hsl],
                    start=True, stop=False,
                )
                nc.tensor.matmul(
                    pps[:],
                    lhsT=stgB[:, scol:scol + P],
                    rhs=wp2b[:, hsl],
                    start=False, stop=True,
                )
                nc.vector.scalar_tensor_tensor(
                    ysb[:, hsl], pps[:], 0.0, yp[:, hsl],
                    AluOpType.bypass, AluOpType.add,
                )
            nc.sync.dma_start(y_d[s * P:(s + 1) * P, :], ysb[:])

        # ---- minimal warm-up: q/k column group 0 + v chunks 0,1 -------------
        for part in ("q", "k"):
            emit_qkv_pair_part(0, part, 0)
        emit_v(0)
        emit_v(1)

        # ---- filler schedule: (t, j, i) -> list of thunks -------------------
        filler = {}

        def add_filler(t, j, i, fn):
            filler.setdefault((t, j, i), []).append(fn)

        # pair 0, j0: rest of pair-0 QKV + v chunks 2..15
        for idx, (prt, nt) in enumerate(
            [("q", 1), ("k", 1), ("q", 2), ("k", 2), ("q", 3), ("k", 3)]
        ):
            add_filler(0, 0, 2 * idx, lambda prt=prt, nt=nt: emit_qkv_pair_part(0, prt, nt))
        _vslots = {1: (2, 3, 4), 3: (5, 6), 5: (7, 8), 7: (9, 10), 9: (11, 12),
                   11: (13, 14), 12: (15,)}
        for i_, ss in _vslots.items():
            for s_ in ss:
                add_filler(0, 0, i_, lambda s_=s_: emit_v(s_))
        # pair 0, j1..j3: pair-1 QKV
        for idx, (prt, nt) in enumerate(
            [("q", 0), ("q", 1), ("q", 2), ("q", 3),
             ("k", 0), ("k", 1), ("k", 2), ("k", 3)]
        ):
            jj, pos = divmod(idx, 3)
            add_filler(0, 1 + jj, 3 + 5 * pos,
                       lambda prt=prt, nt=nt: emit_qkv_pair_part(1, prt, nt))
        # pair 1: pair-2 QKV
        for idx, (prt, nt) in enumerate(
            [("q", 0), ("q", 1), ("q", 2), ("q", 3),
             ("k", 0), ("k", 1), ("k", 2), ("k", 3)]
        ):
            jj, pos = divmod(idx, 2)
            add_filler(1, jj, 3 + 7 * pos,
                       lambda prt=prt, nt=nt: emit_qkv_pair_part(2, prt, nt))
        # pair 2, j1..j3: proj tiles for the previous j's query range;
        # j3 additionally precomputes the pair-0/1 partials of s=12..15
        for jj in range(1, NJ):
            for z in range(4):
                add_filler(2, jj, 3 + 3 * z,
                           lambda s_=4 * (jj - 1) + z: emit_proj_mtile(s_))
        for z in range(4):
            add_filler(2, NJ - 1, 2 + 3 * z,
                       lambda s_=4 * (NJ - 1) + z: emit_proj_partial(s_))

        # ---- attention per (pair, j-tile) -----------------------------------
        for t in range(PAIRS):
            qt_, kt_ = qk_tiles[t]
            at = atpool.tile([P, N], BF16, tag="at", name=f"at{t}")
            at_tiles.append(at)
            for j in range(NJ):
                jsl = slice(j * QT, (j + 1) * QT)
                ua = upool.tile([HD + 1, QT], F32, tag="u", name="ua")
                ub = upool.tile([HD + 1, QT], F32, tag="u", name="ub")
                # U matmuls for chunk i are emitted AFTER chunk i+1's S + exp
                # so the two K=64 row-group S matmuls stay adjacent (they run
                # concurrently at PE array rows 0-63/64-127).  Both heads'
                # scores share one 2-bank PSUM tile so a single [128, 1024]
                # activation serves the chunk (per-instruction ScalarE
                # overhead made two 512-wide activations 45% slower).
                pend = []

                def emit_u(eab, i):
                    nc.tensor.matmul(
                        ua[:], lhsT=v4r[:, i, 2 * t, :], rhs=eab[:, 0:QT],
                        start=(i == 0), stop=(i == NKC - 1),
                    )
                    nc.tensor.matmul(
                        ub[:], lhsT=v4r[:, i, 2 * t + 1, :], rhs=eab[:, QT:2 * QT],
                        start=(i == 0), stop=(i == NKC - 1),
                    )

                for i in range(NKC):
                    sab = spool.tile([P, 2 * QT], F32, tag="s", name="sab")
                    nc.tensor.matmul(
                        sab[:, 0:QT], lhsT=kt_[0:HD, i * P:(i + 1) * P],
                        rhs=qt_[0:HD, jsl], start=True, stop=True,
                    )
                    nc.tensor.matmul(
                        sab[:, QT:2 * QT], lhsT=kt_[HD:P, i * P:(i + 1) * P],
                        rhs=qt_[HD:P, jsl], start=True, stop=True,
                    )
                    eab = epool.tile([P, 2 * QT], BF16, tag="e", name="eab")
                    nc.scalar.activation(eab[:], sab[:], EXP, scale=SCALE)
                    # pipeline depth 2: U(i) lands two chunks after its S, so
                    # the exp latency is hidden even in chunks with no filler
                    if len(pend) == 2:
                        emit_u(*pend.pop(0))
                    pend.append((eab, i))
                    for fn in filler.get((t, j, i), ()):
                        fn()
                for p_ in pend:
                    emit_u(*p_)
                # Drain U psum to SBUF right away so the PSUM slots recycle.
                usa = uspool.tile([HD + 1, QT], F32, tag="us", name="usa")
                nc.vector.tensor_copy(usa[:], ua[:])
                usb = uspool.tile([HD + 1, QT], F32, tag="us", name="usb")
                nc.vector.tensor_copy(usb[:], ub[:])
                # normalize: out = U / rowsum  (rowsum in partition HD).
                # Broadcast the raw rowsum FIRST (gpsimd), then take the
                # reciprocal at full width on the DVE — a [1,512]
                # reciprocal runs on a single DVE lane and is as slow as the
                # broadcast itself.  reciprocal_approx_fast also corrupts
                # data at a non-zero base partition, so the rowsum row is
                # DMA'd down to partition 0 first (gpsimd queue so it never
                # waits behind a y-tile write on the sync queue).
                rsa = rpool.tile([1, QT], F32, tag="rs", name="rsa")
                nc.gpsimd.dma_start(rsa[:], usa[HD:HD + 1, :])
                rba = rbpool.tile([HD, QT], F32, tag="rb", name="rba")
                nc.gpsimd.partition_broadcast(rba[:], rsa[:])
                ra = rbpool.tile([HD, QT], F32, tag="ri", name="ra")
                nc.vector.reciprocal_approx_fast(ra[:], rba[:])
                nc.vector.tensor_mul(at[0:HD, jsl], usa[0:HD, :], ra[:])

                rsb = rpool.tile([1, QT], F32, tag="rs", name="rsb")
                nc.gpsimd.dma_start(rsb[:], usb[HD:HD + 1, :])
                rbb = rbpool.tile([HD, QT], F32, tag="rb", name="rbb")
                nc.gpsimd.partition_broadcast(rbb[:], rsb[:])
                rb_ = rbpool.tile([HD, QT], F32, tag="ri", name="rb_")
                nc.vector.reciprocal_approx_fast(rb_[:], rbb[:])
                # head B: normalize into a bf16 staging tile, then DMA-shift
                # to partitions 64..127 (engines cannot shift partitions).
                stg = rbpool.tile([HD, QT], BF16, tag="st", name="stg")
                nc.vector.tensor_mul(stg[:], usb[0:HD, :], rb_[:])
                if t == PAIRS - 1 and j == NJ - 1:
                    last_stg = stg   # consumed in place by emit_proj_finish
                else:
                    nc.gpsimd.dma_start(at[HD:P, jsl], stg[:])

        # ---- remaining projection (only the pair-2 matmuls + add) ----------
        for s in range(4 * (NJ - 1), NKC):
            emit_proj_finish(s, last_stg)


def build_program():
    nc = bacc.Bacc(
        "TRN2", target_bir_lowering=False, debug=False, num_devices=NCORES
    )
    xT_d = nc.dram_tensor("xT", [DIM, N], BF16, kind="ExternalInput").ap()
    wqkT_d = nc.dram_tensor("wqkT", [DIM, 2 * CH], BF16, kind="ExternalInput").ap()
    wvT_d = nc.dram_tensor("wvT", [DIM, CH], BF16, kind="ExternalInput").ap()
    bqk_d = nc.dram_tensor("bqk", [2 * CH, 1], F32, kind="ExternalInput").ap()
    wpT_d = nc.dram_tensor("wpT", [CH, DIM], BF16, kind="ExternalInput").ap()
    y_d = nc.dram_tensor("y", [N, DIM], BF16, kind="ExternalOutput").ap()
    with tile.TileContext(nc) as tc:
        _emit(tc, xT_d, wqkT_d, wvT_d, bqk_d, wpT_d, y_d)
    nc.compile()
    return nc


def get_program():
    global _PROGRAM
    if _PROGRAM is None:
        _PROGRAM = build_program()
    return _PROGRAM


def make_in_maps(x, Wqkv, bqkv, Wproj):
    import ml_dtypes

    bf16 = ml_dtypes.bfloat16
    x = np.asarray(x, np.float32)
    Wqkv = np.asarray(Wqkv, np.float32)
    bqkv = np.asarray(bqkv, np.float32)
    in_maps = []
    for c in range(NCORES):
        b, g = divmod(c, GPB)
        cs = slice(g * CH, (g + 1) * CH)
        wq = Wqkv[0 * DIM:1 * DIM][cs]
        wk = Wqkv[1 * DIM:2 * DIM][cs]
        wv_ = Wqkv[2 * DIM:3 * DIM][cs]
        in_maps.append({
            "xT": np.ascontiguousarray(x[b].T).astype(bf16),
            "wqkT": np.ascontiguousarray(
                np.concatenate([wq, wk], 0).T
            ).astype(bf16),
            "wvT": np.ascontiguousarray(wv_.T).astype(bf16),
            "bqk": np.concatenate(
                [bqkv[0 * DIM:1 * DIM][cs], bqkv[1 * DIM:2 * DIM][cs]]
            )[:, None].copy(),
            "wpT": np.ascontiguousarray(
                np.asarray(Wproj, np.float32)[:, cs].T
            ).astype(bf16),
        })
    return in_maps


def combine_outputs(per_core_y, bqkv, bproj, Wproj):
    bproj = np.asarray(bproj, np.float32)
    bqkv = np.asarray(bqkv, np.float32)
    Wproj = np.asarray(Wproj, np.float32)
    # exact fold of the v-bias through the projection (attn rows sum to 1)
    bias = bproj + Wproj @ bqkv[2 * DIM:3 * DIM]
    out = np.empty((B, N, DIM), np.float32)
    for b in range(B):
        out[b] = per_core_y[GPB * b] + per_core_y[GPB * b + 1] + bias[None, :]
    return out


def kernel(**inputs):
    ratio = int(np.asarray(inputs.get("ratio", 1)))
    assert ratio == 1, f"kernel specialized for ratio=1, got {ratio}"
    nc = get_program()
    in_maps = make_in_maps(
        inputs["x"], inputs["Wqkv"], inputs["bqkv"], inputs["Wproj"]
    )
    res = run_bass_kernel_spmd(nc, in_maps, list(range(NCORES)))
    ys = [np.asarray(res.results[c]["y"], np.float32) for c in range(NCORES)]
    return combine_outputs(ys, inputs["bqkv"], inputs["bproj"], inputs["Wproj"])


# revision 36
# speedup vs baseline: 1.1789x; 1.1789x over previous
"""Trainium2 Bass kernel for nn_Attention_70557722739202.

Standard MHA block: qkv = x @ Wqkv.T + bqkv; attn = softmax(q k^T / 8);
out = (attn v) @ Wproj.T + bproj, with B=4, N=2048, C=768, H=12, hd=64
(ratio == 1 so the slimmable slicing is identity).

Sharding (8 cores): batch x head-group.  Core c handles batch c//2 and
heads [6*(c%2), 6*(c%2)+6).  Wqkv rows / Wproj cols are sharded by head;
each core emits a partial projection output [2048, 768] and the host sums
the two partials per batch (+ bproj + Wproj @ bv, the exact fold of the
v-bias through the projection).

Rework (425us baseline -> ~300us measured; HW exec time has ~15-18%
run-to-run chip-clock noise, so compare min-of-N):
  - whole datapath in bf16 (PE rate is unchanged vs float32r at >=256
    moving, but SBUF/DMA/DVE traffic and LDWEIGHTS bytes halve); PSUM
    accumulation and the U drains/normalize stay fp32.
  - qkv biases moved off the PE: q/k bias rides the PSUM->SBUF drain as a
    DVE tensor_scalar_add with a per-partition [128,1] bias vector
    (channel-major layout); the v bias is folded on the host into the
    output bias (y += Wproj @ bv exactly, since rows of attn sum to 1).
    This removes 40 K=1 matmuls that each cost a full ~500ns PE pass.
  - both heads' scores share one 2-bank [128,1024] PSUM tile (spool
    bufs=2 = 4 banks, double-buffered so S(i+1) never waits on exp(i)),
    and ONE [128,1024] activation serves the chunk: ScalarE has ~260ns
    fixed per-instruction overhead, so two 512-wide activations are 45%
    slower than one 1024-wide.  PSUM: 4 (S) + 2 (U) + 2 (filler) = 8.
  - startup: single-trigger DMAs (descriptor generation costs ~600ns
    per dma_start), warm-up-critical slices (pair-0 wqk columns, x)
    issued first and split across the sync + scalar queues; only q/k
    column-group 0 + v chunks 0-1 are computed before attention starts,
    everything else runs as PE filler inside the attention chunk loops.
  - tail: proj tiles are emitted as filler per completed j-tile of pair
    2; for s=12..15 the pair-0/1 contributions are precomputed as
    filler and the pair-2 contribution is a split-K=64+64 pair of
    matmuls, head B taken straight from the normalize staging tile so
    the partition-shift DMA (and its ~4us queue drain) stays off the
    critical tail.  After the last normalize only 8 small matmuls +
    DVE adds remain.
  - normalize: broadcast the raw rowsum first (gpsimd), then
    reciprocal_approx_fast at full width on the DVE (a [1,512]
    reciprocal runs on a single DVE lane and is as slow as the
    broadcast itself); normalize-path DMAs ride the gpsimd queue so
    they never wait behind 384KB y-tile writes on the sync queue.
Per-core dataflow is otherwise the baseline's: S.T = k q^T per head via
row-packed K=64 matmuls (2 heads at array rows 0-63/64-127), exp on
ScalarE straight out of PSUM with the 1/8 scale folded in, U.T = [v|1]^T
expS.T accumulated over key chunks (row 64 = softmax denominator), and
the U matmuls of chunk i emitted after chunk i+1's S+exp (software
pipelining that keeps the two row-group S matmuls adjacent).
"""

import os
import sys

for _p in ("/opt/trn_rl_repo",):
    if os.path.isdir(_p) and _p not in sys.path:
        sys.path.insert(0, _p)

import numpy as np

import concourse.bacc as bacc
import concourse.mybir as mybir
import concourse.tile as tile
from concourse.bass_utils import run_bass_kernel_spmd

DIM = 768
NHEADS = 12
B, N = 4, 2048
HD = 64          # head dim
NCORES = 8
HPC = 6          # heads per core
PAIRS = 3        # head pairs per core
GPB = 2          # head groups per batch
CH = HPC * HD    # 384 output channels per core
SCALE = (DIM // NHEADS) ** -0.5
P = 128
QT = 512         # query tile width (1 PSUM bank per S tile)
NJ = N // QT     # 4 query tiles
NKC = N // P     # 16 key chunks
KC = DIM // P    # 6 input-channel chunks
F32 = mybir.dt.float32
F32R = mybir.dt.float32r
BF16 = mybir.dt.bfloat16
EXP = mybir.ActivationFunctionType.Exp

_PROGRAM = None


def _emit(tc, xT_d, wqkT_d, wvT_d, bqk_d, wpT_d, y_d):
    nc = tc.nc

    from contextlib import ExitStack

    with ExitStack() as ctx:
        const = ctx.enter_context(tc.tile_pool(name="const", bufs=1))
        qkpool = ctx.enter_context(tc.tile_pool(name="qkpool", bufs=4))
        atpool = ctx.enter_context(tc.tile_pool(name="atpool", bufs=3))
        epool = ctx.enter_context(tc.tile_pool(name="epool", bufs=4))
        rpool = ctx.enter_context(tc.tile_pool(name="rpool", bufs=2))
        rbpool = ctx.enter_context(tc.tile_pool(name="rbpool", bufs=2))
        uspool = ctx.enter_context(tc.tile_pool(name="uspool", bufs=2))
        ypool = ctx.enter_context(tc.tile_pool(name="ypool", bufs=2))
        spool = ctx.enter_context(tc.tile_pool(name="spool", bufs=2, space="PSUM"))
        upool = ctx.enter_context(tc.tile_pool(name="upool", bufs=2, space="PSUM"))
        fpool = ctx.enter_context(tc.tile_pool(name="fpool", bufs=2, space="PSUM"))

        # ---- resident inputs -------------------------------------------------
        xt = const.tile([P, KC, N], BF16)        # x.T   (in-ch on partitions)
        wqk = const.tile([P, KC, 2 * CH], BF16)  # Wqk.T (in-ch on partitions)
        wv = const.tile([P, KC, CH], BF16)       # Wv.T
        wp = const.tile([P, PAIRS, DIM], BF16)   # Wproj.T slice (ch on part)
        bqk_sb = const.tile([P, 2 * PAIRS], F32)  # col t: q pair t; 3+t: k pair t
        v4 = const.tile([P, NKC, HPC * (HD + 1)], BF16)  # v + ones column

        # Single-trigger transfers (descriptor generation costs ~600ns per
        # dma_start on the issuing engine; per-chunk triggers serialized the
        # stream).  Split across the sync + scalar queues so the two streams
        # move concurrently (~500GB/s aggregate observed).
        nc.sync.dma_start(bqk_sb[:], bqk_d.rearrange("(c p) o -> p (c o)", p=P))
        xtv = xT_d.rearrange("(k p) n -> p k n", p=P)
        wqkv = wqkT_d.rearrange("(k p) n -> p k n", p=P)
        # warm-up-critical first: pair-0's wqk columns + x split across both
        # queues, so the first QKV parts start ~4us sooner; the rest streams
        # behind them.
        nc.scalar.dma_start(wqk[:, :, 0:P], wqkv[:, :, 0:P])
        nc.scalar.dma_start(wqk[:, :, CH:CH + P], wqkv[:, :, CH:CH + P])
        nc.sync.dma_start(xt[:, 0:3, :], xtv[:, 0:3, :])
        nc.scalar.dma_start(xt[:, 3:KC, :], xtv[:, 3:KC, :])
        nc.scalar.dma_start(wqk[:, :, P:CH], wqkv[:, :, P:CH])
        nc.scalar.dma_start(wqk[:, :, CH + P:2 * CH], wqkv[:, :, CH + P:2 * CH])
        nc.sync.dma_start(wv[:], wvT_d.rearrange("(k p) n -> p k n", p=P))
        nc.sync.dma_start(wp[:], wpT_d.rearrange("(k p) n -> p k n", p=P))
        # pair-2 head-B rows of Wproj.T at base partition 0, for the split-K
        # tail matmul against the normalize staging tile (matmul operands
        # must share a base partition; engines can't shift, DMA can)
        wp2b = const.tile([HD, DIM], BF16)
        nc.sync.dma_start(wp2b[:], wp[HD:P, PAIRS - 1, :])
        v4r = v4.rearrange("p n (h c) -> p n h c", c=HD + 1)
        # softmax-rowsum fused column: ones at channel 64 of each head block
        # (memset can't encode float32r — write through a float32 view)
        ones_cols = v4.rearrange(
            "p n (h c) -> p (n h) c", c=HD + 1
        )[:, :, HD:HD + 1]
        nc.vector.memset(ones_cols, 1.0)

        qk_tiles = {}   # t -> (qt, kt)
        at_tiles = []

        def emit_qkv_pair_part(t, part, nt):
            """One eighth of pair t's q.T/k.T: part in {q,k}, nt in {0..3}
            (512-wide column group).  6 matmuls + one biased drain."""
            if t not in qk_tiles:
                qt_ = qkpool.tile([P, N], BF16, tag="qk", name=f"qt{t}")
                kt_ = qkpool.tile([P, N], BF16, tag="qk", name=f"kt{t}")
                qk_tiles[t] = (qt_, kt_)
            qt_, kt_ = qk_tiles[t]
            colofs = t * P if part == "q" else CH + t * P
            bcol = t if part == "q" else PAIRS + t
            dst = qt_ if part == "q" else kt_
            nsl = slice(nt * QT, (nt + 1) * QT)
            ps = fpool.tile([P, QT], F32, tag="f", name="qkps")
            for k in range(KC):
                nc.tensor.matmul(
                    ps[:],
                    lhsT=wqk[:, k, colofs:colofs + P],
                    rhs=xt[:, k, nsl],
                    start=(k == 0), stop=(k == KC - 1),
                )
            nc.vector.tensor_scalar_add(dst[:, nsl], ps[:], bqk_sb[:, bcol:bcol + 1])

        def emit_v(s):
            """v for all 6 heads for sequence chunk s (bias folded on host)."""
            vps = fpool.tile([P, CH], F32, tag="f", name="vps")
            for k in range(KC):
                nc.tensor.matmul(
                    vps[:],
                    lhsT=xt[:, k, s * P:(s + 1) * P],
                    rhs=wv[:, k, :],
                    start=(k == 0), stop=(k == KC - 1),
                )
            nc.vector.tensor_copy(
                v4r[:, s, :, 0:HD],
                vps.rearrange("p (h c) -> p h c", c=HD),
            )

        def emit_proj_mtile(s):
            """Projection for sequence chunk s: y[s*128:(s+1)*128, :]."""
            ysb = ypool.tile([P, DIM], BF16, tag="y", name="ysb")
            for nh in range(2):
                pps = fpool.tile([P, DIM // 2], F32, tag="f", name="pps")
                for t in range(PAIRS):
                    nc.tensor.matmul(
                        pps[:],
                        lhsT=at_tiles[t][:, s * P:(s + 1) * P],
                        rhs=wp[:, t, nh * (DIM // 2):(nh + 1) * (DIM // 2)],
                        start=(t == 0), stop=(t == PAIRS - 1),
                    )
                nc.vector.tensor_copy(
                    ysb[:, nh * (DIM // 2):(nh + 1) * (DIM // 2)], pps[:]
                )
            nc.sync.dma_start(y_d[s * P:(s + 1) * P, :], ysb[:])

        # the last 4 proj tiles gate on the very last normalize; their
        # pair-0/1 contributions are precomputed as filler so only one
        # matmul + an add per half remains after it.
        proj_partials = {}

        def emit_proj_partial(s):
            yp = ypool.tile([P, DIM], F32, tag="yp", name=f"yp{s}", bufs=4)
            for nh in range(2):
                pps = fpool.tile([P, DIM // 2], F32, tag="f", name="pps")
                for t in range(PAIRS - 1):
                    nc.tensor.matmul(
                        pps[:],
                        lhsT=at_tiles[t][:, s * P:(s + 1) * P],
                        rhs=wp[:, t, nh * (DIM // 2):(nh + 1) * (DIM // 2)],
                        start=(t == 0), stop=(t == PAIRS - 2),
                    )
                nc.vector.tensor_copy(
                    yp[:, nh * (DIM // 2):(nh + 1) * (DIM // 2)], pps[:]
                )
            proj_partials[s] = yp

        def emit_proj_finish(s, stgB):
            """Tail of proj tile s: pair-2 contribution split K=64+K=64 so
            head B comes straight from its normalize staging tile — the
            partition-shift DMA (and its queue drain) stays off the tail."""
            from concourse.alu_op_type import AluOpType

            yp = proj_partials[s]
            scol = s * P - 4 * (NJ - 1) * P
            ysb = ypool.tile([P, DIM], BF16, tag="y", name="ysb")
            for nh in range(2):
                hsl = slice(nh * (DIM // 2), (nh + 1) * (DIM // 2))
                pps = fpool.tile([P, DIM // 2], F32, tag="f", name="pps")
                nc.tensor.matmul(
                    pps[:],
                    lhsT=at_tiles[PAIRS - 1][0:HD, s * P:(s + 1) * P],
                    rhs=wp[0:HD, PAIRS - 1, hsl],
                    start=True, stop=False,
                )
                nc.tensor.matmul(
                    pps[:],
                    lhsT=stgB[:, scol:scol + P],
                    rhs=wp2b[:, hsl],
                    start=False, stop=True,
                )
                nc.vector.scalar_tensor_tensor(
                    ysb[:, hsl], pps[:], 0.0, yp[:, hsl],
                    AluOpType.bypass, AluOpType.add,
                )
            nc.sync.dma_start(y_d[s * P:(s + 1) * P, :], ysb[:])

        # ---- minimal warm-up: q/k column group 0 + v chunks 0,1 -------------
        for part in ("q", "k"):
            emit_qkv_pair_part(0, part, 0)
        emit_v(0)
        emit_v(1)

        # ---- filler schedule: (t, j, i) -> list of thunks -------------------
        filler = {}

        def add_filler(t, j, i, fn):
            filler.setdefault((t, j, i), []).append(fn)

        # pair 0, j0: rest of pair-0 QKV + v chunks 2..15
        for idx, (prt, nt) in enumerate(
            [("q", 1), ("k", 1), ("q", 2), ("k", 2), ("q", 3), ("k", 3)]
        ):
            add_filler(0, 0, 2 * idx, lambda prt=prt, nt=nt: emit_qkv_pair_part(0, prt, nt))
        _vslots = {1: (2, 3, 4), 3: (5, 6), 5: (7, 8), 7: (9, 10), 9: (11, 12),
                   11: (13, 14), 12: (15,)}
        for i_, ss in _vslots.items():
            for s_ in ss:
                add_filler(0, 0, i_, lambda s_=s_: emit_v(s_))
        # pair 0, j1..j3: pair-1 QKV
        for idx, (prt, nt) in enumerate(
            [("q", 0), ("q", 1), ("q", 2), ("q", 3),
             ("k", 0), ("k", 1), ("k", 2), ("k", 3)]
        ):
            jj, pos = divmod(idx, 3)
            add_filler(0, 1 + jj, 3 + 5 * pos,
                       lambda prt=prt, nt=nt: emit_qkv_pair_part(1, prt, nt))
        # pair 1: pair-2 QKV
        for idx, (prt, nt) in enumerate(
            [("q", 0), ("q", 1), ("q", 2), ("q", 3),
             ("k", 0), ("k", 1), ("k", 2), ("k", 3)]
        ):
            jj, pos = divmod(idx, 2)
            add_filler(1, jj, 3 + 7 * pos,
                       lambda prt=prt, nt=nt: emit_qkv_pair_part(2, prt, nt))
        # pair 2, j1..j3: proj tiles for the previous j's query range;
        # j3 additionally precomputes the pair-0/1 partials of s=12..15
        for jj in range(1, NJ):
            for z in range(4):
                add_filler(2, jj, 3 + 3 * z,
                           lambda s_=4 * (jj - 1) + z: emit_proj_mtile(s_))
        for z in range(4):
            add_filler(2, NJ - 1, 2 + 3 * z,
                       lambda s_=4 * (NJ - 1) + z: emit_proj_partial(s_))

        # ---- attention per (pair, j-tile) -----------------------------------
        for t in range(PAIRS):
            qt_, kt_ = qk_tiles[t]
            at = atpool.tile([P, N], BF16, tag="at", name=f"at{t}")
            at_tiles.append(at)
            for j in range(NJ):
                jsl = slice(j * QT, (j + 1) * QT)
                ua = upool.tile([HD + 1, QT], F32, tag="u", name="ua")
                ub = upool.tile([HD + 1, QT], F32, tag="u", name="ub")
                # U matmuls for chunk i are emitted AFTER chunk i+1's S + exp
                # so the two K=64 row-group S matmuls stay adjacent (they run
                # concurrently at PE array rows 0-63/64-127).  Both heads'
                # scores share one 2-bank PSUM tile so a single [128, 1024]
                # activation serves the chunk (per-instruction ScalarE
                # overhead made two 512-wide activations 45% slower).
                pend = []

                def emit_u(eab, i):
                    nc.tensor.matmul(
                        ua[:], lhsT=v4r[:, i, 2 * t, :], rhs=eab[:, 0:QT],
                        start=(i == 0), stop=(i == NKC - 1),
                    )
                    nc.tensor.matmul(
                        ub[:], lhsT=v4r[:, i, 2 * t + 1, :], rhs=eab[:, QT:2 * QT],
                        start=(i == 0), stop=(i == NKC - 1),
                    )

                for i in range(NKC):
                    sab = spool.tile([P, 2 * QT], F32, tag="s", name="sab")
                    nc.tensor.matmul(
                        sab[:, 0:QT], lhsT=kt_[0:HD, i * P:(i + 1) * P],
                        rhs=qt_[0:HD, jsl], start=True, stop=True,
                    )
                    nc.tensor.matmul(
                        sab[:, QT:2 * QT], lhsT=kt_[HD:P, i * P:(i + 1) * P],
                        rhs=qt_[HD:P, jsl], start=True, stop=True,
                    )
                    eab = epool.tile([P, 2 * QT], BF16, tag="e", name="eab")
                    nc.scalar.activation(eab[:], sab[:], EXP, scale=SCALE)
                    # pipeline depth 2: U(i) lands two chunks after its S, so
                    # the exp latency is hidden even in chunks with no filler
                    if len(pend) == 2:
                        emit_u(*pend.pop(0))
                    pend.append((eab, i))
                    for fn in filler.get((t, j, i), ()):
                        fn()
                for p_ in pend:
                    emit_u(*p_)
                # Drain U psum to SBUF right away so the PSUM slots recycle.
                usa = uspool.tile([HD + 1, QT], F32, tag="us", name="usa")
                nc.vector.tensor_copy(usa[:], ua[:])
                usb = uspool.tile([HD + 1, QT], F32, tag="us", name="usb")
                nc.vector.tensor_copy(usb[:], ub[:])
                # normalize: out = U / rowsum  (rowsum in partition HD).
                # Broadcast the raw rowsum FIRST (gpsimd), then take the
                # reciprocal at full width on the DVE — a [1,512]
                # reciprocal runs on a single DVE lane and is as slow as the
                # broadcast itself.  reciprocal_approx_fast also corrupts
                # data at a non-zero base partition, so the rowsum row is
                # DMA'd down to partition 0 first (gpsimd queue so it never
                # waits behind a y-tile write on the sync queue).
                rsa = rpool.tile([1, QT], F32, tag="rs", name="rsa")
                nc.gpsimd.dma_start(rsa[:], usa[HD:HD + 1, :])
                rba = rbpool.tile([HD, QT], F32, tag="rb", name="rba")
                nc.gpsimd.partition_broadcast(rba[:], rsa[:])
                ra = rbpool.tile([HD, QT], F32, tag="ri", name="ra")
                nc.vector.reciprocal_approx_fast(ra[:], rba[:])
                nc.vector.tensor_mul(at[0:HD, jsl], usa[0:HD, :], ra[:])

                rsb = rpool.tile([1, QT], F32, tag="rs", name="rsb")
                nc.gpsimd.dma_start(rsb[:], usb[HD:HD + 1, :])
                rbb = rbpool.tile([HD, QT], F32, tag="rb", name="rbb")
                nc.gpsimd.partition_broadcast(rbb[:], rsb[:])
                rb_ = rbpool.tile([HD, QT], F32, tag="ri", name="rb_")
                nc.vector.reciprocal_approx_fast(rb_[:], rbb[:])
                # head B: normalize into a bf16 staging tile, then DMA-shift
                # to partitions 64..127 (engines cannot shift partitions).
                stg = rbpool.tile([HD, QT], BF16, tag="st", name="stg")
                nc.vector.tensor_mul(stg[:], usb[0:HD, :], rb_[:])
                if t == PAIRS - 1 and j == NJ - 1:
                    last_stg = stg   # consumed in place by emit_proj_finish
                else:
                    nc.gpsimd.dma_start(at[HD:P, jsl], stg[:])

        # ---- remaining projection (only the pair-2 matmuls + add) ----------
        for s in range(4 * (NJ - 1), NKC):
            emit_proj_finish(s, last_stg)


def build_program():
    nc = bacc.Bacc(
        "TRN2", target_bir_lowering=False, debug=False, num_devices=NCORES
    )
    xT_d = nc.dram_tensor("xT", [DIM, N], BF16, kind="ExternalInput").ap()
    wqkT_d = nc.dram_tensor("wqkT", [DIM, 2 * CH], BF16, kind="ExternalInput").ap()
    wvT_d = nc.dram_tensor("wvT", [DIM, CH], BF16, kind="ExternalInput").ap()
    bqk_d = nc.dram_tensor("bqk", [2 * CH, 1], F32, kind="ExternalInput").ap()
    wpT_d = nc.dram_tensor("wpT", [CH, DIM], BF16, kind="ExternalInput").ap()
    y_d = nc.dram_tensor("y", [N, DIM], BF16, kind="ExternalOutput").ap()
    with tile.TileContext(nc) as tc:
        _emit(tc, xT_d, wqkT_d, wvT_d, bqk_d, wpT_d, y_d)
    nc.compile()
    return nc


def get_program():
    global _PROGRAM
    if _PROGRAM is None:
        _PROGRAM = build_program()
    return _PROGRAM


def make_in_maps(x, Wqkv, bqkv, Wproj):
    import ml_dtypes

    bf16 = ml_dtypes.bfloat16
    x = np.asarray(x, np.float32)
    Wqkv = np.asarray(Wqkv, np.float32)
    bqkv = np.asarray(bqkv, np.float32)
    in_maps = []
    for c in range(NCORES):
        b, g = divmod(c, GPB)
        cs = slice(g * CH, (g + 1) * CH)
        wq = Wqkv[0 * DIM:1 * DIM][cs]
        wk = Wqkv[1 * DIM:2 * DIM][cs]
        wv_ = Wqkv[2 * DIM:3 * DIM][cs]
        in_maps.append({
            "xT": np.ascontiguousarray(x[b].T).astype(bf16),
            "wqkT": np.ascontiguousarray(
                np.concatenate([wq, wk], 0).T
            ).astype(bf16),
            "wvT": np.ascontiguousarray(wv_.T).astype(bf16),
            "bqk": np.concatenate(
                [bqkv[0 * DIM:1 * DIM][cs], bqkv[1 * DIM:2 * DIM][cs]]
            )[:, None].copy(),
            "wpT": np.ascontiguousarray(
                np.asarray(Wproj, np.float32)[:, cs].T
            ).astype(bf16),
        })
    return in_maps


def combine_outputs(per_core_y, bqkv, bproj, Wproj):
    bproj = np.asarray(bproj, np.float32)
    bqkv = np.asarray(bqkv, np.float32)
    Wproj = np.asarray(Wproj, np.float32)
    # exact fold of the v-bias through the projection (attn rows sum to 1)
    bias = bproj + Wproj @ bqkv[2 * DIM:3 * DIM]
    out = np.empty((B, N, DIM), np.float32)
    for b in range(B):
        out[b] = per_core_y[GPB * b] + per_core_y[GPB * b + 1] + bias[None, :]
    return out


def kernel(**inputs):
    ratio = int(np.asarray(inputs.get("ratio", 1)))
    assert ratio == 1, f"kernel specialized for ratio=1, got {ratio}"
    nc = get_program()
    in_maps = make_in_maps(
        inputs["x"], inputs["Wqkv"], inputs["bqkv"], inputs["Wproj"]
    )
    res = run_bass_kernel_spmd(nc, in_maps, list(range(NCORES)))
    ys = [np.asarray(res.results[c]["y"], np.float32) for c in range(NCORES)]
    return combine_outputs(ys, inputs["bqkv"], inputs["bproj"], inputs["Wproj"])
